# revision 21
# baseline (speedup 1.0000x reference)
"""Multi-head attention (B=2, L=2048, D=1024, H=16) on 8 TRN2 NeuronCores.

Sharding: core c handles batch b = c//4 and head group g = c%4 (4 heads,
256 features). Each core:
  - projects q, k (feature-major) and v (row-major, with a ones-column per
    head so the attn@V matmul emits softmax denominators for free)
  - computes scores^T = K_h Q_h^T tile-wise (keys on partitions), exp on
    ScalarE with the 1/sqrt(DH) scale folded in (no max subtraction: scores
    are ~N(0,1), exp is safe in fp32)
  - accumulates ctx^T = V_h^T P_h in PSUM, normalizes by the denominator row
    via a K=1 broadcast matmul + DVE multiply
  - computes its partial of the output projection out^T = Wo_h ctx^T
The host sums the 4 per-head-group partials per batch and adds bo.
No inter-core communication.
"""

import math
import os
import sys

sys.path.insert(0, "/opt/trn_rl_repo")

import ml_dtypes
import numpy as np

import concourse.bass as bass
import concourse.mybir as mybir
import concourse.tile as tile
from concourse import bacc
from concourse.bass_utils import run_bass_kernel_spmd

B, L, D, H, DH = 2, 2048, 1024, 16, 64
NCORES = 8
HPC = 4                  # heads per core
FPC = HPC * DH           # 256 features per core
ND = D // 128            # 8 contraction tiles
NFT = FPC // 128         # 2 feature tiles for q/k/ctx
NM = L // 128            # 16 key tiles
VW = DH + 1              # 65 = head block width in v (64 feats + ones col)
VROW = HPC * VW          # 260
SCALE = 1.0 / math.sqrt(DH)
CDT = mybir.dt.bfloat16
NP_CDT = ml_dtypes.bfloat16
F32 = mybir.dt.float32
EXP = mybir.ActivationFunctionType.Exp
OUT_NAME = "outT"

_CACHE = {}


def build_nc():
    nc = bacc.Bacc(
        "TRN2",
        target_bir_lowering=False,
        debug=False,
        enable_asserts=False,
        num_devices=NCORES,
    )
    xqT_d = nc.dram_tensor("xqT", [D, L], CDT, kind="ExternalInput")
    xkT_d = nc.dram_tensor("xkT", [D, L], CDT, kind="ExternalInput")
    xvT_d = nc.dram_tensor("xvT", [D, L], CDT, kind="ExternalInput")
    wq_d = nc.dram_tensor("wqT", [D, FPC], CDT, kind="ExternalInput")
    wk_d = nc.dram_tensor("wkT", [D, FPC], CDT, kind="ExternalInput")
    wv_d = nc.dram_tensor("wvT", [D, VROW], CDT, kind="ExternalInput")
    wo_d = nc.dram_tensor("woT", [FPC, D], CDT, kind="ExternalInput")
    bq_d = nc.dram_tensor("bq2", [128, NFT], F32, kind="ExternalInput")
    bk_d = nc.dram_tensor("bk2", [128, NFT], F32, kind="ExternalInput")
    bvb_d = nc.dram_tensor("bvb", [128, VROW], F32, kind="ExternalInput")
    out_d = nc.dram_tensor(OUT_NAME, [D, L], F32, kind="ExternalOutput")

    with tile.TileContext(nc) as tc:
        with tc.tile_pool(name="persist", bufs=1) as pp:
            qT = pp.tile([128, NFT, L], CDT)
            kT = pp.tile([128, NFT, L], CDT)
            vsb = pp.tile([128, NM, VROW], CDT)
            ctxT = pp.tile([128, NFT, L], CDT)
            wo_sb = pp.tile([128, NFT, D], CDT)
            bq_sb = pp.tile([128, NFT], F32)
            bk_sb = pp.tile([128, NFT], F32)
            bvb_sb = pp.tile([128, VROW], F32)
            ones_sb = pp.tile([1, 128], F32)

            # keep the Sync DGE queue clear for the projection inputs the
            # first matmuls need: route weights/biases not needed until much
            # later through the otherwise-idle GpSimd DGE
            nc.vector.memset(ones_sb[:], 1.0)
            nc.gpsimd.dma_start(bq_sb[:], bq_d[:])
            nc.gpsimd.dma_start(bk_sb[:], bk_d[:])
            nc.gpsimd.dma_start(bvb_sb[:], bvb_d[:])
            nc.gpsimd.dma_start(
                wo_sb[:], wo_d.rearrange("(n p) f -> p n f", p=128)
            )

            # ---- Phase A: projections ----
            with (
                tc.tile_pool(name="phA", bufs=1) as pa,
                tc.tile_pool(name="psA", bufs=4, space="PSUM") as psA,
            ):
                wq_sb = pa.tile([128, ND, FPC], CDT)
                wk_sb = pa.tile([128, ND, FPC], CDT)
                wv_sb = pa.tile([128, ND, VROW], CDT)
                xq_sb = pa.tile([128, ND, L], CDT)
                xk_sb = pa.tile([128, ND, L], CDT)
                xv_sb = pa.tile([128, ND, L], CDT)
                xq_r = xqT_d.rearrange("(n p) l -> p n l", p=128)
                xk_r = xkT_d.rearrange("(n p) l -> p n l", p=128)
                xv_r = xvT_d.rearrange("(n p) l -> p n l", p=128)
                wq_r = wq_d.rearrange("(n p) f -> p n f", p=128)
                wk_r = wk_d.rearrange("(n p) f -> p n f", p=128)
                wv_r = wv_d.rearrange("(n p) f -> p n f", p=128)
                for d in range(ND):
                    nc.sync.dma_start(wq_sb[:, d, :], wq_r[:, d, :])
                    nc.sync.dma_start(xq_sb[:, d, :], xq_r[:, d, :])
                for d in range(ND):
                    nc.sync.dma_start(wv_sb[:, d, :], wv_r[:, d, :])
                    nc.sync.dma_start(xv_sb[:, d, :], xv_r[:, d, :])
                for d in range(ND):
                    nc.sync.dma_start(wk_sb[:, d, :], wk_r[:, d, :])
                    nc.sync.dma_start(xk_sb[:, d, :], xk_r[:, d, :])
                # q/k projections d-outer: first matmul only needs d-tile 0
                # (fast ramp, PE warms early); 4 row-chunks share one lhsT
                def proj_qk(x_sb, w_sb, b_sb, dstT, ti):
                    for ft in range(NFT):
                        pss = [
                            psA.tile(
                                [128, 512], F32, tag="projqk",
                                name=f"pjk_{ti}_{ft}_{ch}",
                            )
                            for ch in range(4)
                        ]
                        for d in range(ND):
                            for ch in range(4):
                                nc.tensor.matmul(
                                    pss[ch][:],
                                    w_sb[:, d, ft * 128 : (ft + 1) * 128],
                                    x_sb[:, d, ch * 512 : (ch + 1) * 512],
                                    start=(d == 0),
                                    stop=(d == ND - 1),
                                )
                        for ch in range(4):
                            nc.vector.tensor_scalar_add(
                                dstT[:, ft, ch * 512 : (ch + 1) * 512],
                                pss[ch][:],
                                b_sb[:, ft : ft + 1],
                            )

                proj_qk(xq_sb, wq_sb, bq_sb, qT, 0)
                # v projection: row-major output [keys, feats+ones]
                for kt in range(NM):
                    ps = psA.tile([128, VROW], F32, tag="projv")
                    for d in range(ND):
                        nc.tensor.matmul(
                            ps[:],
                            xv_sb[:, d, kt * 128 : (kt + 1) * 128],
                            wv_sb[:, d, :],
                            start=(d == 0),
                            stop=(d == ND - 1),
                        )
                    nc.vector.tensor_add(vsb[:, kt, :], ps[:], bvb_sb[:])
                # k last: the A->B pool-boundary release then coincides with
                # kT readiness, which the first scores matmuls need anyway
                proj_qk(xk_sb, wk_sb, bk_sb, kT, 1)

            kphase = os.environ.get("KPHASE", "full")
            if kphase == "A":
                with tc.tile_pool(name="dbg", bufs=2) as dbg:
                    for ft in range(NFT):
                        st = dbg.tile([128, L], F32, tag="st")
                        nc.vector.tensor_copy(st[:], qT[:, ft, :])
                        nc.sync.dma_start(out_d[ft * 128 : (ft + 1) * 128, :], st[:])
                        st2 = dbg.tile([128, L], F32, tag="st")
                        nc.vector.tensor_copy(st2[:], kT[:, ft, :])
                        nc.sync.dma_start(
                            out_d[(2 + ft) * 128 : (3 + ft) * 128, :], st2[:]
                        )
                        st3 = dbg.tile([128, L], F32, tag="st")
                        nc.vector.tensor_copy(
                            st3[:, : 7 * VROW],
                            vsb[:, ft * 8 : ft * 8 + 7, :],
                        )
                        nc.vector.memset(st3[:, 7 * VROW :], 0.0)
                        nc.sync.dma_start(
                            out_d[(4 + ft) * 128 : (5 + ft) * 128, :], st3[:]
                        )
            # ---- Phase B: attention ----
            if kphase in ("AB", "full"):
                phase_bc(nc, tc, qT, kT, vsb, ctxT, wo_sb, ones_sb, out_d, kphase)
    nc.compile()
    return nc


def phase_bc(nc, tc, qT, kT, vsb, ctxT, wo_sb, ones_sb, out_d, kphase):
    with (
        tc.tile_pool(name="probs", bufs=48) as pb,
        tc.tile_pool(name="smalls", bufs=2) as sm,
        tc.tile_pool(name="psS", bufs=2, space="PSUM") as psS,
        tc.tile_pool(name="psC", bufs=2, space="PSUM") as psC,
        tc.tile_pool(name="psX", bufs=2, space="PSUM") as psX,
    ):
        def normalize(ctx, hi, hp, lc):
            # approx_fast mis-executes on HW when operands sit at base_partition
            # != 0, so stage the denominator row at partition 0 first
            den = sm.tile([1, 512], F32, tag="den", name=f"den_{hp}_{hi}_{lc}")
            nc.vector.tensor_copy(den[:], ctx[64:65, :])
            rec = sm.tile([1, 512], F32, tag="rec", name=f"rec_{hp}_{hi}_{lc}")
            nc.vector.reciprocal_approx_fast(rec[:], den[:])
            rb_ps = psX.tile([128, 512], F32, tag="acc512", name=f"rb_{hp}_{hi}_{lc}")
            nc.tensor.matmul(rb_ps[:], ones_sb[:], rec[:], start=True, stop=True)
            rb_sb = sm.tile([128, 512], F32, tag="rbsb", name=f"rbsb_{hp}_{hi}_{lc}")
            nc.vector.tensor_copy(rb_sb[:], rb_ps[:])
            po = hi * 64
            nc.vector.tensor_mul(
                ctxT[po : po + 64, hp, lc * 512 : (lc + 1) * 512],
                ctx[0:64, :],
                rb_sb[0:64, :],
            )

        for qh in range(2):
            for hp in range(HPC // 2):
                probs = {}
                # first-half attnV chains run inside the m loop, one per head
                chain = {
                    hi: psC.tile([VW, 512], F32, tag="ctx", name=f"ctx_{qh}_{hp}_{hi}_a")
                    for hi in range(2)
                }
                for m in range(NM):
                    scs = []
                    for hi in range(2):
                        po = hi * 64
                        sc = psS.tile([128, 1024], F32, tag="sc", name=f"sc_{qh}_{hp}_{m}_{hi}")
                        for c2 in range(2):
                            qo = qh * 1024 + c2 * 512
                            nc.tensor.matmul(
                                sc[:, c2 * 512 : (c2 + 1) * 512],
                                kT[po : po + 64, hp, m * 128 : (m + 1) * 128],
                                qT[po : po + 64, hp, qo : qo + 512],
                                start=True,
                                stop=True,
                            )
                        scs.append(sc)
                    for hi, sc in enumerate(scs):
                        pr = pb.tile([128, 1024], CDT, tag="probs", name=f"pr_{qh}_{hp}_{m}_{hi}")
                        nc.scalar.activation(pr[:], sc[:], EXP, scale=SCALE)
                        probs[(hi, m)] = pr
                        h = 2 * hp + hi
                        nc.tensor.matmul(
                            chain[hi][:],
                            vsb[:, m, h * VW : (h + 1) * VW],
                            pr[:, 0:512],
                            start=(m == 0),
                            stop=(m == NM - 1),
                        )
                for hi in range(2):
                    normalize(chain[hi], hi, hp, qh * 2)
                # second-half chains stream from fully materialized probs
                for hi in range(2):
                    h = 2 * hp + hi
                    ctx = psC.tile([VW, 512], F32, tag="ctx", name=f"ctx_{qh}_{hp}_{hi}_b")
                    for m in range(NM):
                        nc.tensor.matmul(
                            ctx[:],
                            vsb[:, m, h * VW : (h + 1) * VW],
                            probs[(hi, m)][:, 512:1024],
                            start=(m == 0),
                            stop=(m == NM - 1),
                        )
                    normalize(ctx, hi, hp, qh * 2 + 1)

            if kphase == "AB":
                continue
            # ---- output projection for the two finished q chunks ----
            for lc in (qh * 2, qh * 2 + 1):
                for ft8 in range(D // 128):
                    ops = psX.tile([128, 512], F32, tag="acc512", name=f"op_{lc}_{ft8}")
                    for d2 in range(NFT):
                        nc.tensor.matmul(
                            ops[:],
                            wo_sb[:, d2, ft8 * 128 : (ft8 + 1) * 128],
                            ctxT[:, d2, lc * 512 : (lc + 1) * 512],
                            start=(d2 == 0),
                            stop=(d2 == NFT - 1),
                        )
                    st = sm.tile([128, 512], F32, tag="ost", bufs=4, name=f"st_{lc}_{ft8}")
                    nc.vector.tensor_copy(st[:], ops[:])
                    nc.gpsimd.dma_start(
                        out_d[ft8 * 128 : (ft8 + 1) * 128, lc * 512 : (lc + 1) * 512],
                        st[:],
                    )

        if kphase == "AB":
            for ft in range(NFT):
                st = sm.tile([128, L], F32, tag="ostage", name=f"dbg_{ft}")
                nc.vector.tensor_copy(st[:], ctxT[:, ft, :])
                nc.sync.dma_start(out_d[ft * 128 : (ft + 1) * 128, :], st[:])


def make_in_maps(Q, K, V, Wq, bq, Wk, bk, Wv, bv, Wo, bo):
    Q = np.asarray(Q, np.float32)
    K = np.asarray(K, np.float32)
    V = np.asarray(V, np.float32)
    xqT = [np.ascontiguousarray(Q[b].T).astype(NP_CDT) for b in range(B)]
    xkT = [np.ascontiguousarray(K[b].T).astype(NP_CDT) for b in range(B)]
    xvT = [np.ascontiguousarray(V[b].T).astype(NP_CDT) for b in range(B)]
    in_maps = []
    for c in range(NCORES):
        b, g = divmod(c, HPC)
        fs = slice(g * FPC, (g + 1) * FPC)
        wqT = np.ascontiguousarray(np.asarray(Wq, np.float32)[fs, :].T).astype(NP_CDT)
        wkT = np.ascontiguousarray(np.asarray(Wk, np.float32)[fs, :].T).astype(NP_CDT)
        # v weights: per-head [64 cols | zero col], bias bcast carries the 1.0
        wv_blk = np.zeros((D, VROW), np.float32)
        bv_blk = np.zeros((VROW,), np.float32)
        wv_slc = np.asarray(Wv, np.float32)[fs, :].T  # [D, 256]
        bv_slc = np.asarray(bv, np.float32)[fs]
        for h in range(HPC):
            wv_blk[:, h * VW : h * VW + DH] = wv_slc[:, h * DH : (h + 1) * DH]
            bv_blk[h * VW : h * VW + DH] = bv_slc[h * DH : (h + 1) * DH]
            bv_blk[h * VW + DH] = 1.0
        woT = np.ascontiguousarray(np.asarray(Wo, np.float32)[:, fs].T).astype(NP_CDT)
        bq2 = np.ascontiguousarray(
            np.asarray(bq, np.float32)[fs].reshape(NFT, 128).T
        )
        bk2 = np.ascontiguousarray(
            np.asarray(bk, np.float32)[fs].reshape(NFT, 128).T
        )
        in_maps.append(
            {
                "xqT": xqT[b],
                "xkT": xkT[b],
                "xvT": xvT[b],
                "wqT": wqT,
                "wkT": wkT,
                "wvT": wv_blk.astype(NP_CDT),
                "woT": woT,
                "bq2": bq2,
                "bk2": bk2,
                "bvb": np.broadcast_to(bv_blk, (128, VROW)).copy(),
            }
        )
    return in_maps


def assemble(results, bo):
    out = np.zeros((B, L, D), np.float32)
    for c in range(NCORES):
        b = c // HPC
        out[b] += results[c][OUT_NAME].T
    out += np.asarray(bo, np.float32)[None, None, :]
    return out


def kernel(Q, K, V, Wq, bq, Wk, bk, Wv, bv, Wo, bo):
    if "nc" not in _CACHE:
        _CACHE["nc"] = build_nc()
    nc = _CACHE["nc"]
    in_maps = make_in_maps(Q, K, V, Wq, bq, Wk, bk, Wv, bv, Wo, bo)
    res = run_bass_kernel_spmd(nc, in_maps, core_ids=list(range(NCORES)))
    return assemble(res.results, bo)
